# revision 11
# baseline (speedup 1.0000x reference)
"""AM-Softmax loss on 8 TRN2 NeuronCores.

Data-parallel over N: each core takes 256 rows of score (256 x 50257 f32),
streams them through SBUF computing rowsum_i = sum_c exp(S * score[i, c])
with a fused ScalarE exp+row-accumulate, then does the tiny label-dependent
tail on-device (labels are in {0, 1}, so the target-logit gather is a
select between columns 0 and 1). Each core emits its 256 per-row L values;
the host concatenates and returns -mean(L).
"""

import numpy as np

import concourse.bass as bass
import concourse.tile as tile
from concourse import bacc, mybir
from concourse.bass_utils import run_bass_kernel_spmd

# Problem constants (hardcoded per spec)
N = 2048
C = 50257
NCORES = 8
R = N // NCORES  # 256 rows per core
S = 30.0
M_S = 0.1
M_L = 0.4

NBLK = R // 128  # 2 row-blocks of 128 partitions

F32 = mybir.dt.float32
AF = mybir.ActivationFunctionType
ALU = mybir.AluOpType
AX = mybir.AxisListType

# Streaming config (tuned via bench.py)
CFG = dict(T=8192, bufs=4, dual_queue=False)


def chunks_for(T):
    return [(c0, min(T, C - c0)) for c0 in range(0, C, T)]


def emit_pass(nc, stream_pool, small_pool, score, lab, out, cfg=None):
    """Emit one full loss pass (streaming exp row-sums + tail)."""
    cfg = {**CFG, **(cfg or {})}
    no_out_dma = cfg.get("no_out_dma", False)
    ln_func = AF.Identity if cfg.get("noln", False) else AF.Ln
    T = cfg["T"]
    chunks = chunks_for(T)
    nchunk = len(chunks)
    assert nchunk <= 16

    acc = small_pool.tile([128, 16 * NBLK], F32)
    sc0 = small_pool.tile([128, NBLK], F32)
    sc1 = small_pool.tile([128, NBLK], F32)
    labt = small_pool.tile([128, NBLK], F32)
    rowsum = small_pool.tile([128, NBLK], F32)
    diff = small_pool.tile([128, NBLK], F32)
    prod = small_pool.tile([128, NBLK], F32)
    target = small_pool.tile([128, NBLK], F32)
    mt = small_pool.tile([128, NBLK], F32)
    tm = small_pool.tile([128, NBLK], F32)
    num = small_pool.tile([128, NBLK], F32)
    expnum = small_pool.tile([128, NBLK], F32)
    expst = small_pool.tile([128, NBLK], F32)
    d2 = small_pool.tile([128, NBLK], F32)
    denom = small_pool.tile([128, NBLK], F32)
    ld = small_pool.tile([128, NBLK], F32)
    L = small_pool.tile([128, NBLK], F32)

    dma_engines = (
        [nc.sync, nc.scalar] if cfg["dual_queue"] else [nc.sync]
    )

    # lab via the SWDGE (gpsimd) queue so the HWDGE FIFO carries only the
    # big streaming loads
    nc.gpsimd.dma_start(
        out=labt[:, 0:NBLK],
        in_=lab.ap().rearrange("(b p) one -> p (b one)", p=128),
    )

    def emit_mid_tail(b):
        # Everything that needs only sc0/sc1/lab for block b — traced
        # mid-stream so DVE/ACT run it while streaming continues.
        c = slice(b, b + 1)
        # target = sc0 + lab * (sc1 - sc0)
        nc.vector.tensor_sub(diff[:, c], sc1[:, c], sc0[:, c])
        nc.vector.tensor_mul(prod[:, c], labt[:, c], diff[:, c])
        nc.vector.tensor_add(target[:, c], sc0[:, c], prod[:, c])
        # m = M_S + lab * (M_L - M_S)
        nc.vector.tensor_scalar(
            mt[:, c], labt[:, c], M_L - M_S, M_S, ALU.mult, ALU.add
        )
        # numerator = S * (target - m)
        nc.vector.tensor_sub(tm[:, c], target[:, c], mt[:, c])
        nc.vector.tensor_scalar_mul(num[:, c], tm[:, c], S)
        nc.scalar.activation(expnum[:, c], tm[:, c], AF.Exp, scale=S)
        nc.scalar.activation(expst[:, c], target[:, c], AF.Exp, scale=S)
        # partial denom (everything but rowsum)
        nc.vector.tensor_sub(d2[:, c], expnum[:, c], expst[:, c])

    def emit_end_tail(b):
        # rowsum-dependent chain for block b + its 512B contiguous output.
        c = slice(b, b + 1)
        # denom = exp(num) - exp(S*target) + rowsum
        nc.vector.tensor_add(denom[:, c], d2[:, c], rowsum[:, c])
        nc.scalar.activation(ld[:, c], denom[:, c], ln_func)
        nc.vector.tensor_sub(L[:, c], num[:, c], ld[:, c])
        if not no_out_dma:
            # block 0's output goes out mid-stream on the SWDGE queue;
            # only block NBLK-1's write sits on the critical tail.
            eng = nc.gpsimd if b < NBLK - 1 else nc.sync
            eng.dma_start(
                out=out[b * 128 : (b + 1) * 128, 0:1], in_=L[:, c]
            )

    for b in range(NBLK):
        for j, (c0, w) in enumerate(chunks):
            t = stream_pool.tile([128, T], F32, tag="stream")
            eng = dma_engines[(b * nchunk + j) % len(dma_engines)]
            eng.dma_start(
                out=t[:, :w],
                in_=score[b * 128 : (b + 1) * 128, c0 : c0 + w],
            )
            if j == 0:
                # grab raw score columns 0,1 before the in-place exp
                nc.vector.tensor_copy(sc0[:, b : b + 1], t[:, 0:1])
                nc.vector.tensor_copy(sc1[:, b : b + 1], t[:, 1:2])
                emit_mid_tail(b)
            # t = exp(S * t); acc col = per-partition row sum of exp
            nc.scalar.activation(
                t[:, :w],
                t[:, :w],
                AF.Exp,
                scale=S,
                accum_out=acc[:, b * 16 + j : b * 16 + j + 1],
            )
        nc.vector.reduce_sum(
            rowsum[:, b : b + 1], acc[:, b * 16 : b * 16 + nchunk], axis=AX.X
        )
        emit_end_tail(b)


def build(m_repeats: int = 1, cfg=None):
    """m_repeats > 1 builds a benchmarking NEFF that runs the whole pass
    M times back-to-back; the graded kernel uses 1."""
    cfg = {**CFG, **(cfg or {})}
    nc = bacc.Bacc(
        "TRN2",
        target_bir_lowering=False,
        debug=False,
        num_devices=NCORES,
    )
    score = nc.dram_tensor("score", [R, C], F32, kind="ExternalInput")
    lab = nc.dram_tensor("lab", [R, 1], F32, kind="ExternalInput")
    out = nc.dram_tensor("out", [R, 1], F32, kind="ExternalOutput")

    with tile.TileContext(nc) as tc:
        with (
            tc.tile_pool(name="stream", bufs=cfg["bufs"]) as stream_pool,
            tc.tile_pool(name="small", bufs=1) as small_pool,
        ):
            for _rep in range(m_repeats):
                emit_pass(nc, stream_pool, small_pool, score, lab, out, cfg)

    nc.compile()
    return nc


def build_loop(m_iters: int, cfg=None):
    """One NEFF running the pass m_iters times via a hardware For_i loop.

    cfg["mode"]: "full" (default) = real pass; "dma" = streaming DMAs only;
    "act" = activations only on resident tiles (scale=0 to stay finite).
    """
    cfg = {**CFG, **(cfg or {})}
    mode = cfg.get("mode", "full")
    nc = bacc.Bacc(
        "TRN2", target_bir_lowering=False, debug=False, num_devices=NCORES
    )
    score = nc.dram_tensor("score", [R, C], F32, kind="ExternalInput")
    lab = nc.dram_tensor("lab", [R, 1], F32, kind="ExternalInput")
    out = nc.dram_tensor("out", [R, 1], F32, kind="ExternalOutput")
    with tile.TileContext(nc) as tc:
        with (
            tc.tile_pool(name="stream", bufs=cfg["bufs"]) as stream_pool,
            tc.tile_pool(name="small", bufs=1) as small_pool,
        ):
            T = cfg["T"]
            chunks = chunks_for(T)
            nchunk = len(chunks)
            if mode == "full":
                with tc.For_i(0, m_iters, 1):
                    emit_pass(nc, stream_pool, small_pool, score, lab, out, cfg)
            elif mode == "dma":
                labt = small_pool.tile([128, NBLK], F32)
                with tc.For_i(0, m_iters, 1):
                    for b in range(NBLK):
                        for c0, w in chunks:
                            t = stream_pool.tile([128, T], F32, tag="stream")
                            nc.sync.dma_start(
                                out=t[:, :w],
                                in_=score[b * 128 : (b + 1) * 128, c0 : c0 + w],
                            )
                for b in range(NBLK):
                    nc.sync.dma_start(
                        out=labt[:, b : b + 1],
                        in_=lab[b * 128 : (b + 1) * 128, 0:1],
                    )
                    nc.sync.dma_start(
                        out=out[b * 128 : (b + 1) * 128, 0:1],
                        in_=labt[:, b : b + 1],
                    )
            elif mode == "stream":
                acc = small_pool.tile([128, 16 * NBLK], F32)
                labt = small_pool.tile([128, NBLK], F32)
                with tc.For_i(0, m_iters, 1):
                    for b in range(NBLK):
                        for j, (c0, w) in enumerate(chunks):
                            t = stream_pool.tile([128, T], F32, tag="stream")
                            nc.sync.dma_start(
                                out=t[:, :w],
                                in_=score[b * 128 : (b + 1) * 128, c0 : c0 + w],
                            )
                            nc.scalar.activation(
                                t[:, :w], t[:, :w], AF.Exp, scale=S,
                                accum_out=acc[:, b * 16 + j : b * 16 + j + 1],
                            )
                for b in range(NBLK):
                    nc.sync.dma_start(
                        out=labt[:, b : b + 1],
                        in_=lab[b * 128 : (b + 1) * 128, 0:1],
                    )
                    nc.sync.dma_start(
                        out=out[b * 128 : (b + 1) * 128, 0:1],
                        in_=labt[:, b : b + 1],
                    )
            elif mode == "act":
                acc = small_pool.tile([128, 16 * NBLK], F32)
                labt = small_pool.tile([128, NBLK], F32)
                res = [stream_pool.tile([128, T], F32, tag=f"res{i}")
                       for i in range(cfg["bufs"])]
                for i, t in enumerate(res):
                    nc.sync.dma_start(
                        out=t[:], in_=score[0:128, i * T : (i + 1) * T]
                    )
                with tc.For_i(0, m_iters, 1):
                    for b in range(NBLK):
                        for j, (c0, w) in enumerate(chunks):
                            t = res[(b * nchunk + j) % len(res)]
                            nc.scalar.activation(
                                t[:, :w], t[:, :w], AF.Exp, scale=0.0,
                                accum_out=acc[:, b * 16 + j : b * 16 + j + 1],
                            )
                for b in range(NBLK):
                    nc.sync.dma_start(
                        out=labt[:, b : b + 1],
                        in_=lab[b * 128 : (b + 1) * 128, 0:1],
                    )
                    nc.sync.dma_start(
                        out=out[b * 128 : (b + 1) * 128, 0:1],
                        in_=labt[:, b : b + 1],
                    )
            else:
                raise ValueError(mode)
    nc.compile()
    return nc


_NC_CACHE = {}


def _get_nc():
    if "nc" not in _NC_CACHE:
        _NC_CACHE["nc"] = build()
    return _NC_CACHE["nc"]


def make_in_maps(score: np.ndarray, labels: np.ndarray):
    score = np.asarray(score, dtype=np.float32)
    labf = np.asarray(labels, dtype=np.float32).reshape(N, 1)
    in_maps = []
    for c in range(NCORES):
        in_maps.append(
            {
                "score": np.ascontiguousarray(score[c * R : (c + 1) * R]),
                "lab": np.ascontiguousarray(labf[c * R : (c + 1) * R]),
            }
        )
    return in_maps


def combine(results) -> np.ndarray:
    Ls = np.concatenate([np.asarray(r["out"]).reshape(R) for r in results])
    return np.asarray(-Ls.astype(np.float64).mean(), dtype=np.float32)


def kernel(score: np.ndarray, labels: np.ndarray) -> np.ndarray:
    nc = _get_nc()
    res = run_bass_kernel_spmd(nc, make_in_maps(score, labels), core_ids=list(range(NCORES)))
    return combine(res.results)


# revision 20
# speedup vs baseline: 1.2468x; 1.2468x over previous
"""AM-Softmax loss on 8 TRN2 NeuronCores.

Data-parallel over N: each core takes 256 rows of score (256 x 50257 f32),
streams them through SBUF computing rowsum_i = sum_c exp(S * score[i, c])
with a fused ScalarE exp+row-accumulate, then does the tiny label-dependent
tail on-device (labels are in {0, 1}, so the target-logit gather is a
select between columns 0 and 1). Each core reduces its per-row L values to
2 partial sums on the TensorEngine; the host sums the 16 partials and
returns -sum/N.
"""

import numpy as np

import concourse.bass as bass
import concourse.tile as tile
from concourse import bacc, mybir
from concourse.bass_utils import run_bass_kernel_spmd

# Problem constants (hardcoded per spec)
N = 2048
C = 50257
NCORES = 8
R = N // NCORES  # 256 rows per core
S = 30.0
M_S = 0.1
M_L = 0.4

NBLK = R // 128  # 2 row-blocks of 128 partitions

F32 = mybir.dt.float32
AF = mybir.ActivationFunctionType
ALU = mybir.AluOpType
AX = mybir.AxisListType

# Streaming config (tuned via bench.py: T=11520/bufs=3 beat 8192/4 and
# 23040/2 in same-process A/B; 5 chunks x 2 blocks = 10 streaming DMAs)
CFG = dict(T=11520, bufs=3, dual_queue=False)


def chunks_for(T):
    return [(c0, min(T, C - c0)) for c0 in range(0, C, T)]


def emit_pass(nc, stream_pool, small_pool, psum_pool, score, lab, out, cfg=None):
    """Emit one full loss pass (streaming exp row-sums + tail).

    Device output: out[b, 0] = sum_p L[b*128 + p]  (NBLK partial sums;
    the partition-dim reduction runs on the otherwise-idle TensorEngine
    so the final DRAM write is a 2-descriptor DMA instead of a 128-line
    scatter, which measures ~9 us slower).
    """
    cfg = {**CFG, **(cfg or {})}
    no_out_dma = cfg.get("no_out_dma", False)
    ln_func = AF.Identity if cfg.get("noln", False) else AF.Ln
    T = cfg["T"]
    chunks = chunks_for(T)
    nchunk = len(chunks)
    assert nchunk <= 16

    acc = small_pool.tile([128, 16 * NBLK], F32)
    sc0 = small_pool.tile([128, NBLK], F32)
    sc1 = small_pool.tile([128, NBLK], F32)
    labt = small_pool.tile([128, NBLK], F32)
    rowsum = small_pool.tile([128, NBLK], F32)
    diff = small_pool.tile([128, NBLK], F32)
    prod = small_pool.tile([128, NBLK], F32)
    target = small_pool.tile([128, NBLK], F32)
    mt = small_pool.tile([128, NBLK], F32)
    tm = small_pool.tile([128, NBLK], F32)
    num = small_pool.tile([128, NBLK], F32)
    expnum = small_pool.tile([128, NBLK], F32)
    expst = small_pool.tile([128, NBLK], F32)
    d2 = small_pool.tile([128, NBLK], F32)
    denom = small_pool.tile([128, NBLK], F32)
    ld = small_pool.tile([128, NBLK], F32)
    L = small_pool.tile([128, NBLK], F32)
    ones = small_pool.tile([128, 1], F32)
    osum = small_pool.tile([NBLK, 1], F32)
    psum = psum_pool.tile([NBLK, 1], F32)

    nc.gpsimd.memset(ones[:], 1.0)

    dma_engines = (
        [nc.sync, nc.scalar] if cfg["dual_queue"] else [nc.sync]
    )

    # lab via the SWDGE (gpsimd) queue so the HWDGE FIFO carries only the
    # big streaming loads
    nc.gpsimd.dma_start(
        out=labt[:, 0:NBLK],
        in_=lab.ap().rearrange("(b p) one -> p (b one)", p=128),
    )

    def emit_mid_tail(b):
        # Everything that needs only sc0/sc1/lab for block b — traced
        # mid-stream so DVE/ACT run it while streaming continues.
        c = slice(b, b + 1)
        # target = sc0 + lab * (sc1 - sc0)
        nc.vector.tensor_sub(diff[:, c], sc1[:, c], sc0[:, c])
        nc.vector.tensor_mul(prod[:, c], labt[:, c], diff[:, c])
        nc.vector.tensor_add(target[:, c], sc0[:, c], prod[:, c])
        # m = M_S + lab * (M_L - M_S)
        nc.vector.tensor_scalar(
            mt[:, c], labt[:, c], M_L - M_S, M_S, ALU.mult, ALU.add
        )
        # numerator = S * (target - m)
        nc.vector.tensor_sub(tm[:, c], target[:, c], mt[:, c])
        nc.vector.tensor_scalar_mul(num[:, c], tm[:, c], S)
        nc.scalar.activation(expnum[:, c], tm[:, c], AF.Exp, scale=S)
        nc.scalar.activation(expst[:, c], target[:, c], AF.Exp, scale=S)
        # partial denom (everything but rowsum)
        nc.vector.tensor_sub(d2[:, c], expnum[:, c], expst[:, c])

    def emit_end_tail(b):
        # rowsum-dependent chain for block b
        c = slice(b, b + 1)
        # denom = exp(num) - exp(S*target) + rowsum
        nc.vector.tensor_add(denom[:, c], d2[:, c], rowsum[:, c])
        nc.scalar.activation(ld[:, c], denom[:, c], ln_func)
        nc.vector.tensor_sub(L[:, c], num[:, c], ld[:, c])
        if b == NBLK - 1 and not no_out_dma:
            # osum[b] = sum_p L[p, b] via TensorE; 2-line output DMA
            nc.tensor.matmul(psum[:, 0:1], L[:, 0:NBLK], ones[:, 0:1])
            nc.vector.tensor_copy(osum[:, 0:1], psum[:, 0:1])
            nc.sync.dma_start(out=out[0:NBLK, 0:1], in_=osum[:, 0:1])

    for b in range(NBLK):
        for j, (c0, w) in enumerate(chunks):
            t = stream_pool.tile([128, T], F32, tag="stream")
            eng = dma_engines[(b * nchunk + j) % len(dma_engines)]
            eng.dma_start(
                out=t[:, :w],
                in_=score[b * 128 : (b + 1) * 128, c0 : c0 + w],
            )
            if j == 0:
                # grab raw score columns 0,1 before the in-place exp
                nc.vector.tensor_copy(sc0[:, b : b + 1], t[:, 0:1])
                nc.vector.tensor_copy(sc1[:, b : b + 1], t[:, 1:2])
                emit_mid_tail(b)
            # t = exp(S * t); acc col = per-partition row sum of exp
            nc.scalar.activation(
                t[:, :w],
                t[:, :w],
                AF.Exp,
                scale=S,
                accum_out=acc[:, b * 16 + j : b * 16 + j + 1],
            )
        nc.vector.reduce_sum(
            rowsum[:, b : b + 1], acc[:, b * 16 : b * 16 + nchunk], axis=AX.X
        )
        emit_end_tail(b)


def build(m_repeats: int = 1, cfg=None):
    """m_repeats > 1 builds a benchmarking NEFF that runs the whole pass
    M times back-to-back; the graded kernel uses 1."""
    cfg = {**CFG, **(cfg or {})}
    nc = bacc.Bacc(
        "TRN2",
        target_bir_lowering=False,
        debug=False,
        num_devices=NCORES,
    )
    score = nc.dram_tensor("score", [R, C], F32, kind="ExternalInput")
    lab = nc.dram_tensor("lab", [R, 1], F32, kind="ExternalInput")
    out = nc.dram_tensor("out", [NBLK, 1], F32, kind="ExternalOutput")

    with tile.TileContext(nc) as tc:
        with (
            tc.tile_pool(name="stream", bufs=cfg["bufs"]) as stream_pool,
            tc.tile_pool(name="small", bufs=1) as small_pool,
            tc.tile_pool(name="psum", bufs=1, space="PSUM") as psum_pool,
        ):
            for _rep in range(m_repeats):
                emit_pass(
                    nc, stream_pool, small_pool, psum_pool, score, lab, out, cfg
                )

    nc.compile()
    return nc


def build_loop(m_iters: int, cfg=None):
    """One NEFF running the pass m_iters times via a hardware For_i loop.

    cfg["mode"]: "full" (default) = real pass; "dma" = streaming DMAs only;
    "act" = activations only on resident tiles (scale=0 to stay finite).
    """
    cfg = {**CFG, **(cfg or {})}
    mode = cfg.get("mode", "full")
    nc = bacc.Bacc(
        "TRN2", target_bir_lowering=False, debug=False, num_devices=NCORES
    )
    score = nc.dram_tensor("score", [R, C], F32, kind="ExternalInput")
    lab = nc.dram_tensor("lab", [R, 1], F32, kind="ExternalInput")
    out = nc.dram_tensor("out", [NBLK, 1], F32, kind="ExternalOutput")
    with tile.TileContext(nc) as tc:
        with (
            tc.tile_pool(name="stream", bufs=cfg["bufs"]) as stream_pool,
            tc.tile_pool(name="small", bufs=1) as small_pool,
            tc.tile_pool(name="psum", bufs=1, space="PSUM") as psum_pool,
        ):
            T = cfg["T"]
            chunks = chunks_for(T)
            nchunk = len(chunks)
            if mode == "full":
                with tc.For_i(0, m_iters, 1):
                    emit_pass(
                        nc, stream_pool, small_pool, psum_pool,
                        score, lab, out, cfg,
                    )
            elif mode == "dma":
                labt = small_pool.tile([128, NBLK], F32)
                with tc.For_i(0, m_iters, 1):
                    for b in range(NBLK):
                        for c0, w in chunks:
                            t = stream_pool.tile([128, T], F32, tag="stream")
                            nc.sync.dma_start(
                                out=t[:, :w],
                                in_=score[b * 128 : (b + 1) * 128, c0 : c0 + w],
                            )
                nc.sync.dma_start(
                    out=labt[:, 0:1], in_=lab[0:128, 0:1]
                )
                nc.sync.dma_start(
                    out=out[0:NBLK, 0:1], in_=labt[0:NBLK, 0:1]
                )
            elif mode == "stream":
                acc = small_pool.tile([128, 16 * NBLK], F32)
                labt = small_pool.tile([128, NBLK], F32)
                with tc.For_i(0, m_iters, 1):
                    for b in range(NBLK):
                        for j, (c0, w) in enumerate(chunks):
                            t = stream_pool.tile([128, T], F32, tag="stream")
                            nc.sync.dma_start(
                                out=t[:, :w],
                                in_=score[b * 128 : (b + 1) * 128, c0 : c0 + w],
                            )
                            nc.scalar.activation(
                                t[:, :w], t[:, :w], AF.Exp, scale=S,
                                accum_out=acc[:, b * 16 + j : b * 16 + j + 1],
                            )
                nc.sync.dma_start(
                    out=labt[:, 0:1], in_=lab[0:128, 0:1]
                )
                nc.sync.dma_start(
                    out=out[0:NBLK, 0:1], in_=labt[0:NBLK, 0:1]
                )
            elif mode == "act":
                acc = small_pool.tile([128, 16 * NBLK], F32)
                labt = small_pool.tile([128, NBLK], F32)
                res = [stream_pool.tile([128, T], F32, tag=f"res{i}")
                       for i in range(cfg["bufs"])]
                for i, t in enumerate(res):
                    nc.sync.dma_start(
                        out=t[:], in_=score[0:128, i * T : (i + 1) * T]
                    )
                with tc.For_i(0, m_iters, 1):
                    for b in range(NBLK):
                        for j, (c0, w) in enumerate(chunks):
                            t = res[(b * nchunk + j) % len(res)]
                            nc.scalar.activation(
                                t[:, :w], t[:, :w], AF.Exp, scale=0.0,
                                accum_out=acc[:, b * 16 + j : b * 16 + j + 1],
                            )
                nc.sync.dma_start(
                    out=labt[:, 0:1], in_=lab[0:128, 0:1]
                )
                nc.sync.dma_start(
                    out=out[0:NBLK, 0:1], in_=labt[0:NBLK, 0:1]
                )
            else:
                raise ValueError(mode)
    nc.compile()
    return nc


_NC_CACHE = {}


def _get_nc():
    if "nc" not in _NC_CACHE:
        _NC_CACHE["nc"] = build()
    return _NC_CACHE["nc"]


def make_in_maps(score: np.ndarray, labels: np.ndarray):
    score = np.asarray(score, dtype=np.float32)
    labf = np.asarray(labels, dtype=np.float32).reshape(N, 1)
    in_maps = []
    for c in range(NCORES):
        in_maps.append(
            {
                "score": np.ascontiguousarray(score[c * R : (c + 1) * R]),
                "lab": np.ascontiguousarray(labf[c * R : (c + 1) * R]),
            }
        )
    return in_maps


def combine(results) -> np.ndarray:
    # each core's "out" holds NBLK partial sums of L over its 128-row blocks
    total = sum(
        np.asarray(r["out"]).astype(np.float64).sum() for r in results
    )
    return np.asarray(-total / N, dtype=np.float32)


def kernel(score: np.ndarray, labels: np.ndarray) -> np.ndarray:
    nc = _get_nc()
    res = run_bass_kernel_spmd(nc, make_in_maps(score, labels), core_ids=list(range(NCORES)))
    return combine(res.results)


# revision 28
# speedup vs baseline: 1.5287x; 1.2260x over previous
"""AM-Softmax loss on 8 TRN2 NeuronCores.

Data-parallel over N: each core takes 256 rows of score (256 x 50257 f32),
streams them through SBUF computing rowsum_i = sum_c exp(S * score[i, c])
with a fused ScalarE exp+row-accumulate, then does the tiny label-dependent
tail on-device (labels are in {0, 1}, so the target-logit gather is a
select between columns 0 and 1). Each core reduces its per-row L values to
2 partial sums on the TensorEngine; the host sums the 16 partials and
returns -sum/N.
"""

import numpy as np

import concourse.bass as bass
import concourse.tile as tile
from concourse import bacc, mybir
from concourse.bass_utils import run_bass_kernel_spmd

# Problem constants (hardcoded per spec)
N = 2048
C = 50257
NCORES = 8
R = N // NCORES  # 256 rows per core
S = 30.0
M_S = 0.1
M_L = 0.4

NBLK = R // 128  # 2 row-blocks of 128 partitions

F32 = mybir.dt.float32
AF = mybir.ActivationFunctionType
ALU = mybir.AluOpType
AX = mybir.AxisListType

# Streaming config (tuned via bench.py). fp16=True stores score as float16
# (host-side cast in make_in_maps): halves HBM traffic; ScalarE computes in
# fp32 internally. exp is computed biased as exp(S*x - EXPB) so the in-place
# fp16 activation output stays in range whether the HW accumulator sums pre-
# or post-conversion values; the e^EXPB rescale is folded into the denom op.
# Measured end-to-end rel err vs the f32 reference: 2.9e-6.
CFG = dict(T=23040, bufs=3, dual_queue=False, fp16=True)
EXPB = 20.0


def chunks_for(T, rem_first=False):
    ch = [(c0, min(T, C - c0)) for c0 in range(0, C, T)]
    if rem_first and len(ch) > 1:
        ch = ch[-1:] + ch[:-1]
    return ch


def emit_pass(nc, stream_pool, small_pool, psum_pool, score, lab, out, cfg=None):
    """Emit one full loss pass (streaming exp row-sums + tail).

    Device output: out[b, 0] = sum_p L[b*128 + p]  (NBLK partial sums;
    the partition-dim reduction runs on the otherwise-idle TensorEngine
    so the final DRAM write is a 2-descriptor DMA instead of a 128-line
    scatter, which measures ~9 us slower).
    """
    cfg = {**CFG, **(cfg or {})}
    no_out_dma = cfg.get("no_out_dma", False)
    ln_func = AF.Identity if cfg.get("noln", False) else AF.Ln
    fp16 = cfg.get("fp16", False)
    sdt = mybir.dt.float16 if fp16 else F32
    T = cfg["T"]
    # remainder chunk first: ScalarE (the fp16-mode bottleneck) starts ~15us
    # earlier on the short chunk while the first big transfer is in flight
    chunks = chunks_for(T, rem_first=fp16)
    nchunk = len(chunks)
    assert nchunk <= 16

    acc = small_pool.tile([128, 16 * NBLK], F32)
    sc0 = small_pool.tile([128, NBLK], F32)
    sc1 = small_pool.tile([128, NBLK], F32)
    labt = small_pool.tile([128, NBLK], F32)
    rowsum = small_pool.tile([128, NBLK], F32)
    diff = small_pool.tile([128, NBLK], F32)
    prod = small_pool.tile([128, NBLK], F32)
    target = small_pool.tile([128, NBLK], F32)
    mt = small_pool.tile([128, NBLK], F32)
    tm = small_pool.tile([128, NBLK], F32)
    num = small_pool.tile([128, NBLK], F32)
    expnum = small_pool.tile([128, NBLK], F32)
    expst = small_pool.tile([128, NBLK], F32)
    d2 = small_pool.tile([128, NBLK], F32)
    denom = small_pool.tile([128, NBLK], F32)
    ld = small_pool.tile([128, NBLK], F32)
    L = small_pool.tile([128, NBLK], F32)
    ones = small_pool.tile([128, 1], F32)
    osum = small_pool.tile([NBLK, 1], F32)
    psum = psum_pool.tile([NBLK, 1], F32)
    if fp16:
        expb = small_pool.tile([128, 1], F32)
        nc.gpsimd.memset(expb[:], -EXPB)

    dma_engines = (
        [nc.sync, nc.scalar] if cfg["dual_queue"] else [nc.sync]
    )

    if cfg.get("no_gpsimd", False):
        # keep POOL fully idle: its dge_drain at barriers/kernel exit is
        # expensive; the 1KB lab load rides the sync FIFO ahead of streaming
        nc.vector.memset(ones[:], 1.0)
        nc.sync.dma_start(
            out=labt[:, 0:NBLK],
            in_=lab.ap().rearrange("(b p) one -> p (b one)", p=128),
        )
    else:
        nc.gpsimd.memset(ones[:], 1.0)
        # lab via the SWDGE (gpsimd) queue so the HWDGE FIFO carries only
        # the big streaming loads
        nc.gpsimd.dma_start(
            out=labt[:, 0:NBLK],
            in_=lab.ap().rearrange("(b p) one -> p (b one)", p=128),
        )

    def emit_mid_tail(b):
        # Everything that needs only sc0/sc1/lab for block b — traced
        # mid-stream so DVE/ACT run it while streaming continues.
        c = slice(b, b + 1)
        # target = sc0 + lab * (sc1 - sc0)
        nc.vector.tensor_sub(diff[:, c], sc1[:, c], sc0[:, c])
        nc.vector.tensor_mul(prod[:, c], labt[:, c], diff[:, c])
        nc.vector.tensor_add(target[:, c], sc0[:, c], prod[:, c])
        # m = M_S + lab * (M_L - M_S)
        nc.vector.tensor_scalar(
            mt[:, c], labt[:, c], M_L - M_S, M_S, ALU.mult, ALU.add
        )
        # numerator = S * (target - m)
        nc.vector.tensor_sub(tm[:, c], target[:, c], mt[:, c])
        nc.vector.tensor_scalar_mul(num[:, c], tm[:, c], S)
        nc.scalar.activation(expnum[:, c], tm[:, c], AF.Exp, scale=S)
        nc.scalar.activation(expst[:, c], target[:, c], AF.Exp, scale=S)
        # partial denom (everything but rowsum)
        nc.vector.tensor_sub(d2[:, c], expnum[:, c], expst[:, c])

    def emit_end_tail(b):
        # rowsum-dependent chain for block b
        c = slice(b, b + 1)
        # denom = exp(num) - exp(S*target) + rowsum  (undo the exp bias)
        if fp16:
            nc.vector.scalar_tensor_tensor(
                denom[:, c], rowsum[:, c], float(np.exp(EXPB)), d2[:, c],
                ALU.mult, ALU.add,
            )
        else:
            nc.vector.tensor_add(denom[:, c], d2[:, c], rowsum[:, c])
        nc.scalar.activation(ld[:, c], denom[:, c], ln_func)
        nc.vector.tensor_sub(L[:, c], num[:, c], ld[:, c])
        if b == NBLK - 1 and not no_out_dma:
            # osum[b] = sum_p L[p, b] via TensorE; 2-line output DMA
            nc.tensor.matmul(psum[:, 0:1], L[:, 0:NBLK], ones[:, 0:1])
            nc.vector.tensor_copy(osum[:, 0:1], psum[:, 0:1])
            nc.sync.dma_start(out=out[0:NBLK, 0:1], in_=osum[:, 0:1])

    for b in range(NBLK):
        grabbed = False
        for j, (c0, w) in enumerate(chunks):
            t = stream_pool.tile([128, T], sdt, tag="stream")
            eng = dma_engines[(b * nchunk + j) % len(dma_engines)]
            eng.dma_start(
                out=t[:, :w],
                in_=score[b * 128 : (b + 1) * 128, c0 : c0 + w],
            )
            if c0 == 0 and not grabbed:
                # grab raw score columns 0,1 before the in-place exp
                nc.vector.tensor_copy(sc0[:, b : b + 1], t[:, 0:1])
                nc.vector.tensor_copy(sc1[:, b : b + 1], t[:, 1:2])
                emit_mid_tail(b)
                grabbed = True
            # t = exp(S*t [- EXPB]); acc col = per-partition row sum
            nc.scalar.activation(
                t[:, :w],
                t[:, :w],
                AF.Exp,
                scale=S,
                bias=expb[:, 0:1] if fp16 else 0.0,
                accum_out=acc[:, b * 16 + j : b * 16 + j + 1],
            )
        nc.vector.reduce_sum(
            rowsum[:, b : b + 1], acc[:, b * 16 : b * 16 + nchunk], axis=AX.X
        )
        emit_end_tail(b)


def build(m_repeats: int = 1, cfg=None):
    """m_repeats > 1 builds a benchmarking NEFF that runs the whole pass
    M times back-to-back; the graded kernel uses 1."""
    cfg = {**CFG, **(cfg or {})}
    nc = bacc.Bacc(
        "TRN2",
        target_bir_lowering=False,
        debug=False,
        num_devices=NCORES,
    )
    sdt = mybir.dt.float16 if cfg.get("fp16", False) else F32
    score = nc.dram_tensor("score", [R, C], sdt, kind="ExternalInput")
    lab = nc.dram_tensor("lab", [R, 1], F32, kind="ExternalInput")
    out = nc.dram_tensor("out", [NBLK, 1], F32, kind="ExternalOutput")

    with tile.TileContext(nc) as tc:
        with (
            tc.tile_pool(name="stream", bufs=cfg["bufs"]) as stream_pool,
            tc.tile_pool(name="small", bufs=1) as small_pool,
            tc.tile_pool(name="psum", bufs=1, space="PSUM") as psum_pool,
        ):
            for _rep in range(m_repeats):
                emit_pass(
                    nc, stream_pool, small_pool, psum_pool, score, lab, out, cfg
                )

    nc.compile()
    return nc


def build_loop(m_iters: int, cfg=None):
    """One NEFF running the pass m_iters times via a hardware For_i loop.

    cfg["mode"]: "full" (default) = real pass; "dma" = streaming DMAs only;
    "act" = activations only on resident tiles (scale=0 to stay finite).
    """
    cfg = {**CFG, **(cfg or {})}
    mode = cfg.get("mode", "full")
    nc = bacc.Bacc(
        "TRN2", target_bir_lowering=False, debug=False, num_devices=NCORES
    )
    sdt = mybir.dt.float16 if cfg.get("fp16", False) else F32
    score = nc.dram_tensor("score", [R, C], sdt, kind="ExternalInput")
    lab = nc.dram_tensor("lab", [R, 1], F32, kind="ExternalInput")
    out = nc.dram_tensor("out", [NBLK, 1], F32, kind="ExternalOutput")
    with tile.TileContext(nc) as tc:
        with (
            tc.tile_pool(name="stream", bufs=cfg["bufs"]) as stream_pool,
            tc.tile_pool(name="small", bufs=1) as small_pool,
            tc.tile_pool(name="psum", bufs=1, space="PSUM") as psum_pool,
        ):
            T = cfg["T"]
            sdt_l = mybir.dt.float16 if cfg.get("fp16", False) else F32
            chunks = chunks_for(T)
            nchunk = len(chunks)
            if mode == "full":
                with tc.For_i(0, m_iters, 1):
                    emit_pass(
                        nc, stream_pool, small_pool, psum_pool,
                        score, lab, out, cfg,
                    )
            elif mode == "dma":
                labt = small_pool.tile([128, NBLK], F32)
                with tc.For_i(0, m_iters, 1):
                    for b in range(NBLK):
                        for c0, w in chunks:
                            t = stream_pool.tile([128, T], sdt_l, tag="stream")
                            nc.sync.dma_start(
                                out=t[:, :w],
                                in_=score[b * 128 : (b + 1) * 128, c0 : c0 + w],
                            )
                nc.sync.dma_start(
                    out=labt[:, 0:1], in_=lab[0:128, 0:1]
                )
                nc.sync.dma_start(
                    out=out[0:NBLK, 0:1], in_=labt[0:NBLK, 0:1]
                )
            elif mode == "stream":
                acc = small_pool.tile([128, 16 * NBLK], F32)
                labt = small_pool.tile([128, NBLK], F32)
                with tc.For_i(0, m_iters, 1):
                    for b in range(NBLK):
                        for j, (c0, w) in enumerate(chunks):
                            t = stream_pool.tile([128, T], sdt_l, tag="stream")
                            nc.sync.dma_start(
                                out=t[:, :w],
                                in_=score[b * 128 : (b + 1) * 128, c0 : c0 + w],
                            )
                            nc.scalar.activation(
                                t[:, :w], t[:, :w], AF.Exp, scale=S,
                                accum_out=acc[:, b * 16 + j : b * 16 + j + 1],
                            )
                nc.sync.dma_start(
                    out=labt[:, 0:1], in_=lab[0:128, 0:1]
                )
                nc.sync.dma_start(
                    out=out[0:NBLK, 0:1], in_=labt[0:NBLK, 0:1]
                )
            elif mode == "act":
                acc = small_pool.tile([128, 16 * NBLK], F32)
                labt = small_pool.tile([128, NBLK], F32)
                res = [stream_pool.tile([128, T], F32, tag=f"res{i}")
                       for i in range(cfg["bufs"])]
                for i, t in enumerate(res):
                    nc.sync.dma_start(
                        out=t[:], in_=score[0:128, i * T : (i + 1) * T]
                    )
                with tc.For_i(0, m_iters, 1):
                    for b in range(NBLK):
                        for j, (c0, w) in enumerate(chunks):
                            t = res[(b * nchunk + j) % len(res)]
                            nc.scalar.activation(
                                t[:, :w], t[:, :w], AF.Exp, scale=0.0,
                                accum_out=acc[:, b * 16 + j : b * 16 + j + 1],
                            )
                nc.sync.dma_start(
                    out=labt[:, 0:1], in_=lab[0:128, 0:1]
                )
                nc.sync.dma_start(
                    out=out[0:NBLK, 0:1], in_=labt[0:NBLK, 0:1]
                )
            else:
                raise ValueError(mode)
    nc.compile()
    return nc


_NC_CACHE = {}


def _get_nc():
    if "nc" not in _NC_CACHE:
        _NC_CACHE["nc"] = build()
    return _NC_CACHE["nc"]


def make_in_maps(score: np.ndarray, labels: np.ndarray):
    sdtype = np.float16 if CFG.get("fp16", False) else np.float32
    score = np.asarray(score).astype(sdtype)
    labf = np.asarray(labels, dtype=np.float32).reshape(N, 1)
    in_maps = []
    for c in range(NCORES):
        in_maps.append(
            {
                "score": np.ascontiguousarray(score[c * R : (c + 1) * R]),
                "lab": np.ascontiguousarray(labf[c * R : (c + 1) * R]),
            }
        )
    return in_maps


def combine(results) -> np.ndarray:
    # each core's "out" holds NBLK partial sums of L over its 128-row blocks
    total = sum(
        np.asarray(r["out"]).astype(np.float64).sum() for r in results
    )
    return np.asarray(-total / N, dtype=np.float32)


def kernel(score: np.ndarray, labels: np.ndarray) -> np.ndarray:
    nc = _get_nc()
    res = run_bass_kernel_spmd(nc, make_in_maps(score, labels), core_ids=list(range(NCORES)))
    return combine(res.results)


# revision 30
# speedup vs baseline: 1.5463x; 1.0116x over previous
"""AM-Softmax loss on 8 TRN2 NeuronCores.

Data-parallel over N: each core takes 256 rows of score (256 x 50257 f32),
streams them through SBUF computing rowsum_i = sum_c exp(S * score[i, c])
with a fused ScalarE exp+row-accumulate, then does the tiny label-dependent
tail on-device (labels are in {0, 1}, so the target-logit gather is a
select between columns 0 and 1). Each core reduces its per-row L values to
2 partial sums on the TensorEngine; the host sums the 16 partials and
returns -sum/N.
"""

import numpy as np

import concourse.bass as bass
import concourse.tile as tile
from concourse import bacc, mybir
from concourse.bass_utils import run_bass_kernel_spmd

# Problem constants (hardcoded per spec)
N = 2048
C = 50257
NCORES = 8
R = N // NCORES  # 256 rows per core
S = 30.0
M_S = 0.1
M_L = 0.4

NBLK = R // 128  # 2 row-blocks of 128 partitions

F32 = mybir.dt.float32
AF = mybir.ActivationFunctionType
ALU = mybir.AluOpType
AX = mybir.AxisListType

# Streaming config (tuned via bench.py). fp16=True stores score as float16
# (host-side cast in make_in_maps): halves HBM traffic; ScalarE computes in
# fp32 internally. exp is computed biased as exp(S*x - EXPB) so the in-place
# fp16 activation output stays in range whether the HW accumulator sums pre-
# or post-conversion values; the e^EXPB rescale is folded into the denom op.
# Measured end-to-end rel err vs the f32 reference: 2.9e-6.
CFG = dict(T=16384, bufs=3, dual_queue=False, fp16=True)
EXPB = 20.0


def chunks_for(T, head_first=False):
    if head_first:
        # tiny head chunk at c0=0 (contains the label columns), then full-T
        # chunks: ScalarE and the label-dependent tail start ~3us in instead
        # of waiting for a big transfer
        w0 = C - ((C - 1) // T) * T
        return [(0, w0)] + [(c0, T) for c0 in range(w0, C, T)]
    return [(c0, min(T, C - c0)) for c0 in range(0, C, T)]


def emit_pass(nc, stream_pool, small_pool, psum_pool, score, lab, out, cfg=None):
    """Emit one full loss pass (streaming exp row-sums + tail).

    Device output: out[b, 0] = sum_p L[b*128 + p]  (NBLK partial sums;
    the partition-dim reduction runs on the otherwise-idle TensorEngine
    so the final DRAM write is a 2-descriptor DMA instead of a 128-line
    scatter, which measures ~9 us slower).
    """
    cfg = {**CFG, **(cfg or {})}
    no_out_dma = cfg.get("no_out_dma", False)
    ln_func = AF.Identity if cfg.get("noln", False) else AF.Ln
    fp16 = cfg.get("fp16", False)
    sdt = mybir.dt.float16 if fp16 else F32
    T = cfg["T"]
    chunks = chunks_for(T, head_first=fp16)
    nchunk = len(chunks)
    assert nchunk <= 16

    acc = small_pool.tile([128, 16 * NBLK], F32)
    sc0 = small_pool.tile([128, NBLK], F32)
    sc1 = small_pool.tile([128, NBLK], F32)
    labt = small_pool.tile([128, NBLK], F32)
    rowsum = small_pool.tile([128, NBLK], F32)
    diff = small_pool.tile([128, NBLK], F32)
    prod = small_pool.tile([128, NBLK], F32)
    target = small_pool.tile([128, NBLK], F32)
    mt = small_pool.tile([128, NBLK], F32)
    tm = small_pool.tile([128, NBLK], F32)
    num = small_pool.tile([128, NBLK], F32)
    expnum = small_pool.tile([128, NBLK], F32)
    expst = small_pool.tile([128, NBLK], F32)
    d2 = small_pool.tile([128, NBLK], F32)
    denom = small_pool.tile([128, NBLK], F32)
    ld = small_pool.tile([128, NBLK], F32)
    L = small_pool.tile([128, NBLK], F32)
    ones = small_pool.tile([128, 1], F32)
    osum = small_pool.tile([NBLK, 1], F32)
    psum = psum_pool.tile([NBLK, 1], F32)
    if fp16:
        expb = small_pool.tile([128, 1], F32)
        nc.gpsimd.memset(expb[:], -EXPB)

    dma_engines = (
        [nc.sync, nc.scalar] if cfg["dual_queue"] else [nc.sync]
    )

    if cfg.get("no_gpsimd", False):
        # keep POOL fully idle: its dge_drain at barriers/kernel exit is
        # expensive; the 1KB lab load rides the sync FIFO ahead of streaming
        nc.vector.memset(ones[:], 1.0)
        nc.sync.dma_start(
            out=labt[:, 0:NBLK],
            in_=lab.ap().rearrange("(b p) one -> p (b one)", p=128),
        )
    else:
        nc.gpsimd.memset(ones[:], 1.0)
        # lab via the SWDGE (gpsimd) queue so the HWDGE FIFO carries only
        # the big streaming loads
        nc.gpsimd.dma_start(
            out=labt[:, 0:NBLK],
            in_=lab.ap().rearrange("(b p) one -> p (b one)", p=128),
        )

    def emit_mid_tail(b):
        # Everything that needs only sc0/sc1/lab for block b — traced
        # mid-stream so DVE/ACT run it while streaming continues.
        c = slice(b, b + 1)
        # target = sc0 + lab * (sc1 - sc0)
        nc.vector.tensor_sub(diff[:, c], sc1[:, c], sc0[:, c])
        nc.vector.tensor_mul(prod[:, c], labt[:, c], diff[:, c])
        nc.vector.tensor_add(target[:, c], sc0[:, c], prod[:, c])
        # m = M_S + lab * (M_L - M_S)
        nc.vector.tensor_scalar(
            mt[:, c], labt[:, c], M_L - M_S, M_S, ALU.mult, ALU.add
        )
        # numerator = S * (target - m)
        nc.vector.tensor_sub(tm[:, c], target[:, c], mt[:, c])
        nc.vector.tensor_scalar_mul(num[:, c], tm[:, c], S)
        nc.scalar.activation(expnum[:, c], tm[:, c], AF.Exp, scale=S)
        nc.scalar.activation(expst[:, c], target[:, c], AF.Exp, scale=S)
        # partial denom (everything but rowsum)
        nc.vector.tensor_sub(d2[:, c], expnum[:, c], expst[:, c])

    def emit_end_tail(b):
        # rowsum-dependent chain for block b
        c = slice(b, b + 1)
        # denom = exp(num) - exp(S*target) + rowsum  (undo the exp bias)
        if fp16:
            nc.vector.scalar_tensor_tensor(
                denom[:, c], rowsum[:, c], float(np.exp(EXPB)), d2[:, c],
                ALU.mult, ALU.add,
            )
        else:
            nc.vector.tensor_add(denom[:, c], d2[:, c], rowsum[:, c])
        nc.scalar.activation(ld[:, c], denom[:, c], ln_func)
        nc.vector.tensor_sub(L[:, c], num[:, c], ld[:, c])
        if b == NBLK - 1 and not no_out_dma:
            # osum[b] = sum_p L[p, b] via TensorE; 2-line output DMA
            nc.tensor.matmul(psum[:, 0:1], L[:, 0:NBLK], ones[:, 0:1])
            nc.vector.tensor_copy(osum[:, 0:1], psum[:, 0:1])
            nc.sync.dma_start(out=out[0:NBLK, 0:1], in_=osum[:, 0:1])

    for b in range(NBLK):
        grabbed = False
        for j, (c0, w) in enumerate(chunks):
            t = stream_pool.tile([128, T], sdt, tag="stream")
            eng = dma_engines[(b * nchunk + j) % len(dma_engines)]
            eng.dma_start(
                out=t[:, :w],
                in_=score[b * 128 : (b + 1) * 128, c0 : c0 + w],
            )
            if c0 == 0 and not grabbed:
                # grab raw score columns 0,1 before the in-place exp
                nc.vector.tensor_copy(sc0[:, b : b + 1], t[:, 0:1])
                nc.vector.tensor_copy(sc1[:, b : b + 1], t[:, 1:2])
                emit_mid_tail(b)
                grabbed = True
            # t = exp(S*t [- EXPB]); acc col = per-partition row sum
            nc.scalar.activation(
                t[:, :w],
                t[:, :w],
                AF.Exp,
                scale=S,
                bias=expb[:, 0:1] if fp16 else 0.0,
                accum_out=acc[:, b * 16 + j : b * 16 + j + 1],
            )
        nc.vector.reduce_sum(
            rowsum[:, b : b + 1], acc[:, b * 16 : b * 16 + nchunk], axis=AX.X
        )
        emit_end_tail(b)


def build(m_repeats: int = 1, cfg=None):
    """m_repeats > 1 builds a benchmarking NEFF that runs the whole pass
    M times back-to-back; the graded kernel uses 1."""
    cfg = {**CFG, **(cfg or {})}
    nc = bacc.Bacc(
        "TRN2",
        target_bir_lowering=False,
        debug=False,
        num_devices=NCORES,
    )
    sdt = mybir.dt.float16 if cfg.get("fp16", False) else F32
    score = nc.dram_tensor("score", [R, C], sdt, kind="ExternalInput")
    lab = nc.dram_tensor("lab", [R, 1], F32, kind="ExternalInput")
    out = nc.dram_tensor("out", [NBLK, 1], F32, kind="ExternalOutput")

    with tile.TileContext(nc) as tc:
        with (
            tc.tile_pool(name="stream", bufs=cfg["bufs"]) as stream_pool,
            tc.tile_pool(name="small", bufs=1) as small_pool,
            tc.tile_pool(name="psum", bufs=1, space="PSUM") as psum_pool,
        ):
            for _rep in range(m_repeats):
                emit_pass(
                    nc, stream_pool, small_pool, psum_pool, score, lab, out, cfg
                )

    nc.compile()
    return nc


def build_loop(m_iters: int, cfg=None):
    """One NEFF running the pass m_iters times via a hardware For_i loop.

    cfg["mode"]: "full" (default) = real pass; "dma" = streaming DMAs only;
    "act" = activations only on resident tiles (scale=0 to stay finite).
    """
    cfg = {**CFG, **(cfg or {})}
    mode = cfg.get("mode", "full")
    nc = bacc.Bacc(
        "TRN2", target_bir_lowering=False, debug=False, num_devices=NCORES
    )
    sdt = mybir.dt.float16 if cfg.get("fp16", False) else F32
    score = nc.dram_tensor("score", [R, C], sdt, kind="ExternalInput")
    lab = nc.dram_tensor("lab", [R, 1], F32, kind="ExternalInput")
    out = nc.dram_tensor("out", [NBLK, 1], F32, kind="ExternalOutput")
    with tile.TileContext(nc) as tc:
        with (
            tc.tile_pool(name="stream", bufs=cfg["bufs"]) as stream_pool,
            tc.tile_pool(name="small", bufs=1) as small_pool,
            tc.tile_pool(name="psum", bufs=1, space="PSUM") as psum_pool,
        ):
            T = cfg["T"]
            sdt_l = mybir.dt.float16 if cfg.get("fp16", False) else F32
            chunks = chunks_for(T)
            nchunk = len(chunks)
            if mode == "full":
                with tc.For_i(0, m_iters, 1):
                    emit_pass(
                        nc, stream_pool, small_pool, psum_pool,
                        score, lab, out, cfg,
                    )
            elif mode == "dma":
                labt = small_pool.tile([128, NBLK], F32)
                with tc.For_i(0, m_iters, 1):
                    for b in range(NBLK):
                        for c0, w in chunks:
                            t = stream_pool.tile([128, T], sdt_l, tag="stream")
                            nc.sync.dma_start(
                                out=t[:, :w],
                                in_=score[b * 128 : (b + 1) * 128, c0 : c0 + w],
                            )
                nc.sync.dma_start(
                    out=labt[:, 0:1], in_=lab[0:128, 0:1]
                )
                nc.sync.dma_start(
                    out=out[0:NBLK, 0:1], in_=labt[0:NBLK, 0:1]
                )
            elif mode == "stream":
                acc = small_pool.tile([128, 16 * NBLK], F32)
                labt = small_pool.tile([128, NBLK], F32)
                with tc.For_i(0, m_iters, 1):
                    for b in range(NBLK):
                        for j, (c0, w) in enumerate(chunks):
                            t = stream_pool.tile([128, T], sdt_l, tag="stream")
                            nc.sync.dma_start(
                                out=t[:, :w],
                                in_=score[b * 128 : (b + 1) * 128, c0 : c0 + w],
                            )
                            nc.scalar.activation(
                                t[:, :w], t[:, :w], AF.Exp, scale=S,
                                accum_out=acc[:, b * 16 + j : b * 16 + j + 1],
                            )
                nc.sync.dma_start(
                    out=labt[:, 0:1], in_=lab[0:128, 0:1]
                )
                nc.sync.dma_start(
                    out=out[0:NBLK, 0:1], in_=labt[0:NBLK, 0:1]
                )
            elif mode == "act":
                acc = small_pool.tile([128, 16 * NBLK], F32)
                labt = small_pool.tile([128, NBLK], F32)
                res = [stream_pool.tile([128, T], F32, tag=f"res{i}")
                       for i in range(cfg["bufs"])]
                for i, t in enumerate(res):
                    nc.sync.dma_start(
                        out=t[:], in_=score[0:128, i * T : (i + 1) * T]
                    )
                with tc.For_i(0, m_iters, 1):
                    for b in range(NBLK):
                        for j, (c0, w) in enumerate(chunks):
                            t = res[(b * nchunk + j) % len(res)]
                            nc.scalar.activation(
                                t[:, :w], t[:, :w], AF.Exp, scale=0.0,
                                accum_out=acc[:, b * 16 + j : b * 16 + j + 1],
                            )
                nc.sync.dma_start(
                    out=labt[:, 0:1], in_=lab[0:128, 0:1]
                )
                nc.sync.dma_start(
                    out=out[0:NBLK, 0:1], in_=labt[0:NBLK, 0:1]
                )
            else:
                raise ValueError(mode)
    nc.compile()
    return nc


_NC_CACHE = {}


def _get_nc():
    if "nc" not in _NC_CACHE:
        _NC_CACHE["nc"] = build()
    return _NC_CACHE["nc"]


def make_in_maps(score: np.ndarray, labels: np.ndarray):
    sdtype = np.float16 if CFG.get("fp16", False) else np.float32
    score = np.asarray(score).astype(sdtype)
    labf = np.asarray(labels, dtype=np.float32).reshape(N, 1)
    in_maps = []
    for c in range(NCORES):
        in_maps.append(
            {
                "score": np.ascontiguousarray(score[c * R : (c + 1) * R]),
                "lab": np.ascontiguousarray(labf[c * R : (c + 1) * R]),
            }
        )
    return in_maps


def combine(results) -> np.ndarray:
    # each core's "out" holds NBLK partial sums of L over its 128-row blocks
    total = sum(
        np.asarray(r["out"]).astype(np.float64).sum() for r in results
    )
    return np.asarray(-total / N, dtype=np.float32)


def kernel(score: np.ndarray, labels: np.ndarray) -> np.ndarray:
    nc = _get_nc()
    res = run_bass_kernel_spmd(nc, make_in_maps(score, labels), core_ids=list(range(NCORES)))
    return combine(res.results)
